# revision 30
# baseline (speedup 1.0000x reference)
"""LlamaAttention (B=1, S=2048, H=4096, 32 heads / 8 KV heads) on 8 TRN2 NeuronCores.

Sharding: tensor-parallel over heads. Core c owns Q heads [4c, 4c+4) and KV head c
(Wq/Wk/Wv column shards, Wo row shard). Each core computes a full [S, H] partial
output in bf16; the host sums the 8 partials (the all-reduce for row-sharded Wo).

vs baseline (894us -> ~404us):
- all matmul operands bf16 (half DMA/SBUF traffic, same PE rate as fp32r);
  error 8e-3 vs the 2e-2 gate. All weights + rope tables SBUF-resident
  (baseline re-streamed Wq 4x); xt block-resident, next block prefetched as
  soon as the last projection pass has read the buffer.
- per-head projection passes cut concurrent PSUM banks so attention gets
  pt x2 + ot x2 banks; out-projection runs as "filler" matmuls interleaved
  into the NEXT block's attention loops + drains, hiding every softmax tail
  (gpsimd partition-reduce 3.5us + reciprocal_approx_fast + normalize) and
  keeping the PE HAM clock at 8/8 (baseline lost 300us to 1.2 GHz throttle).
- qpass(h+1)+rope issued before attn_head(h) so rope lands on the DVE queue
  ahead of head h's softmax sums; filler PSUM-evacs routed ScalarE/VectorE
  by phase so neither engine's backlog stalls the ps_o bank rotation.
- diagonal causal tiles computed at partial width (queries below the band
  see nothing); dummy warm-up matmuls spin the PE clock up during the
  initial DMA fill; 6 filler groups reserved to cover the last head's tail.
"""

import numpy as np

HIDDEN = 4096
N_HEADS = 32
N_KV = 8
HD = 128
S = 2048
N_CORES = 8
HPC = N_HEADS // N_CORES          # 4 Q heads per core
DQ = HPC * HD                     # 512 q columns per core
ROPE_BASE = 10000.0
SCALE = 1.0 / float(np.sqrt(HD))

NBLK = S // 512                   # 4 sq blocks of 512
NSK = S // 128                    # 16 sk tiles of 128
KT = HIDDEN // 128                # 32 contraction tiles

_CACHE = {}


def _build():
    import concourse.bass as bass
    import concourse.tile as tile
    from concourse import bacc, mybir
    import concourse.bass_isa as bass_isa

    f32 = mybir.dt.float32
    bf = mybir.dt.bfloat16
    EXP = mybir.ActivationFunctionType.Exp
    CPY = mybir.ActivationFunctionType.Copy
    ADD = bass_isa.ReduceOp.add

    nc = bacc.Bacc("TRN2", target_bir_lowering=False, debug=False,
                   num_devices=N_CORES)

    xt_d = nc.dram_tensor("xt", [HIDDEN, S], bf, kind="ExternalInput").ap()
    wq_d = nc.dram_tensor("wq", [HIDDEN, DQ], bf, kind="ExternalInput").ap()
    wk_d = nc.dram_tensor("wk", [HIDDEN, HD], bf, kind="ExternalInput").ap()
    wv_d = nc.dram_tensor("wv", [HIDDEN, HD], bf, kind="ExternalInput").ap()
    wo_d = nc.dram_tensor("wo", [DQ, HIDDEN], bf, kind="ExternalInput").ap()
    cos_d = nc.dram_tensor("cosT", [HD, S], bf, kind="ExternalInput").ap()
    sin_d = nc.dram_tensor("sinS", [HD, S], bf, kind="ExternalInput").ap()
    msk_d = nc.dram_tensor("masks", [128, 896], bf, kind="ExternalInput").ap()
    idn_d = nc.dram_tensor("ident", [128, 128], bf, kind="ExternalInput").ap()
    out_d = nc.dram_tensor("out", [S, HIDDEN], bf, kind="ExternalOutput").ap()

    with tile.TileContext(nc) as tc:
        from contextlib import ExitStack
        with ExitStack() as ctx:
            ep = ctx.enter_context
            consts = ep(tc.tile_pool(name="consts", bufs=1))
            main = ep(tc.tile_pool(name="main", bufs=1))
            tmp_pool = ep(tc.tile_pool(name="tmpp", bufs=2))
            et_pool = ep(tc.tile_pool(name="etp", bufs=8))
            sums_pool = ep(tc.tile_pool(name="sumsp", bufs=3))
            sbc_pool = ep(tc.tile_pool(name="sbcp", bufs=3))
            vt_pool = ep(tc.tile_pool(name="vtp", bufs=2))
            osb_pool = ep(tc.tile_pool(name="osbp", bufs=4))
            ps_kv = ep(tc.tile_pool(name="pskv", bufs=1, space="PSUM"))
            ps_q = ep(tc.tile_pool(name="psq", bufs=1, space="PSUM"))
            ps_pt = ep(tc.tile_pool(name="pspt", bufs=2, space="PSUM"))
            ps_ot = ep(tc.tile_pool(name="psot", bufs=2, space="PSUM"))
            ps_o = ep(tc.tile_pool(name="pso", bufs=2, space="PSUM"))

            # ---- resident constants / weights (order = DMA issue order) ----
            wk_s = consts.tile([128, KT, HD], bf)
            wv_s = consts.tile([128, KT, HD], bf)
            wq_s = consts.tile([128, KT, DQ], bf)
            wo_s = consts.tile([128, HPC, HIDDEN], bf)
            cosT = consts.tile([HD, S], bf)
            sinS = consts.tile([HD, S], bf)
            masks = consts.tile([128, 896], bf)
            ident = consts.tile([128, 128], bf)
            xt_s = main.tile([128, KT, 512], bf)
            kt = main.tile([128, S], bf)
            v_sb = main.tile([128, NSK, 128], bf)
            qt = main.tile([128, HPC, 512], bf)
            at = main.tile([128, HPC, S], bf)

            wk_r = wk_d.rearrange("(kt p) m -> p kt m", p=128)
            wv_r = wv_d.rearrange("(kt p) m -> p kt m", p=128)
            wq_r = wq_d.rearrange("(kt p) m -> p kt m", p=128)
            wo_r = wo_d.rearrange("(hh p) m -> p hh m", p=128)

            # K weights + first xt block first so the K pass starts ASAP;
            # wq/wv next (Q0/V pass dependencies), rope tables and Wo later.
            xt_r = xt_d.rearrange("(kt p) s -> p kt s", p=128)
            for q in range(4):
                ksl = slice(q * 8, (q + 1) * 8)
                nc.sync.dma_start(out=wk_s[:, ksl, :], in_=wk_r[:, ksl, :])
            # interleaved so each consumer starts as early as possible:
            # K pass <- wk + xt; ropes <- tables; Q0 <- wq chunks; V <- wv
            for q in range(4):
                ksl = slice(q * 4, (q + 1) * 4)
                nc.sync.dma_start(out=xt_s[:, ksl, :],
                                  in_=xt_r[:, ksl, 0:512])
            nc.sync.dma_start(out=cosT, in_=cos_d)
            nc.sync.dma_start(out=sinS, in_=sin_d)
            nc.sync.dma_start(out=masks, in_=msk_d)
            nc.sync.dma_start(out=ident, in_=idn_d)
            for q in range(2):
                ksl = slice(q * 8, (q + 1) * 8)
                nc.sync.dma_start(out=wq_s[:, ksl, :], in_=wq_r[:, ksl, :])
            for q in range(4, 8):
                ksl = slice(q * 4, (q + 1) * 4)
                nc.sync.dma_start(out=xt_s[:, ksl, :],
                                  in_=xt_r[:, ksl, 0:512])
            for q in range(2, 4):
                ksl = slice(q * 8, (q + 1) * 8)
                nc.sync.dma_start(out=wq_s[:, ksl, :], in_=wq_r[:, ksl, :])
            for q in range(4):
                ksl = slice(q * 8, (q + 1) * 8)
                nc.sync.dma_start(out=wv_s[:, ksl, :], in_=wv_r[:, ksl, :])
            for q in range(4):
                nsl = slice(q * 1024, (q + 1) * 1024)
                nc.sync.dma_start(out=wo_s[:, :, nsl], in_=wo_r[:, :, nsl])

            # PE warm-up: dummy matmuls on an un-initialized tile get the HAM
            # clock gate to 8/8 before the first real matmul arrives.
            wrm = main.tile([128, 512], bf)
            nc.vector.memset(wrm[:], 0.0)
            for w in range(16):
                wps = ps_pt.tile([128, 512], f32, tag="pt", name="wps")
                nc.tensor.matmul(wps[:], wrm[:, 0:128], wrm[:],
                                 start=True, stop=True)

            # ---- out-projection filler machinery (n-pair granularity) ----
            pend = []
            done = [0]

            def filler_group(dve_ok=True, split_dma=False):
                """One filler unit: out rows m*128..+128, cols np2*1024..+1024
                (8 matmuls -> 2 PSUM groups -> one [128,1024] bf16 store).
                Evac engine choice matters: inside attention loops ScalarE is
                busy with exp (use DVE for one half); in the drains the DVE
                holds the softmax tail (recip/at-mul) so use ScalarE only."""
                if not pend:
                    return
                m, np2 = pend.pop(0)
                osb = osb_pool.tile([128, 1024], bf, tag="osb", name="osb")
                for half in range(2):
                    n = np2 * 2 + half
                    o_ps = ps_o.tile([128, 512], f32, tag="ops", name="ops")
                    for hh in range(HPC):
                        nc.tensor.matmul(o_ps[:],
                                         at[:, hh, m * 128:(m + 1) * 128],
                                         wo_s[:, hh, n * 512:(n + 1) * 512],
                                         start=(hh == 0), stop=(hh == HPC - 1))
                    if half == 1 and dve_ok:
                        nc.vector.tensor_copy(osb[:, 512:1024], o_ps[:])
                    else:
                        nc.scalar.activation(osb[:, half * 512:(half + 1) * 512],
                                             o_ps[:], CPY)
                    if split_dma:
                        nc.sync.dma_start(
                            out=out_d[m * 128:(m + 1) * 128,
                                      n * 512:(n + 1) * 512],
                            in_=osb[:, half * 512:(half + 1) * 512])
                if not split_dma:
                    nc.sync.dma_start(
                        out=out_d[m * 128:(m + 1) * 128,
                                  np2 * 1024:(np2 + 1) * 1024],
                        in_=osb[:])
                done[0] += 1

            # ---- per-block building blocks ----
            def rope(ps, dst, sl):
                """dst = rope(ps) (bf16 out) using cosT/sinS tables."""
                t = tmp_pool.tile([128, 512], f32, tag="ropet", name="ropet")
                nc.vector.tensor_mul(t[0:64, :], ps[64:128, :], sinS[0:64, sl])
                nc.vector.tensor_mul(t[64:128, :], ps[0:64, :], sinS[64:128, sl])
                nc.vector.tensor_mul(dst, ps[:], cosT[:, sl])
                nc.vector.tensor_add(dst, dst, t[:])

            def kpass():
                k_ps = ps_kv.tile([128, 512], f32, tag="kv", name="kps")
                for k in range(KT):
                    nc.tensor.matmul(k_ps[:], wk_s[:, k, :], xt_s[:, k, :],
                                     start=(k == 0), stop=(k == KT - 1))
                return k_ps

            def qpass(h):
                q_ps = ps_q.tile([128, 512], f32, tag="qps", name="qps")
                for k in range(KT):
                    nc.tensor.matmul(q_ps[:], wq_s[:, k, h * 128:(h + 1) * 128],
                                     xt_s[:, k, :],
                                     start=(k == 0), stop=(k == KT - 1))
                return q_ps

            def vpass(blk):
                v_ps = ps_kv.tile([128, 512], f32, tag="kv", name="vps")
                for k in range(KT):
                    nc.tensor.matmul(v_ps[:], wv_s[:, k, :], xt_s[:, k, :],
                                     start=(k == 0), stop=(k == KT - 1))
                vt_sb = vt_pool.tile([128, 512], bf, tag="vt", name="vt")
                nc.scalar.activation(vt_sb[:], v_ps[:], CPY)
                vtr = ps_kv.tile([128, 4, 128], bf, tag="kv", name="vtr")
                for t in range(4):
                    nc.tensor.transpose(vtr[:, t, :],
                                        vt_sb[:, t * 128:(t + 1) * 128],
                                        ident[:])
                nc.scalar.activation(v_sb[:, blk * 4:(blk + 1) * 4, :], vtr[:],
                                     CPY)

            def attn_head(h, blk, use_filler):
                lo = blk * 512
                nsk = 4 * (blk + 1)
                ot_ps = ps_ot.tile([128, 512], f32, tag="ot", name="otps")
                sums = sums_pool.tile([128, 512], f32, tag="sums", name="sums")
                for i in range(nsk):
                    off = i - 4 * blk
                    # diagonal-band tiles: queries below off*128 see nothing
                    c0 = off * 128 if off > 0 else 0
                    qs = slice(c0, 512)
                    pt = ps_pt.tile([128, 512], f32, tag="pt", name="pt")
                    nc.tensor.matmul(pt[:, qs], kt[:, i * 128:(i + 1) * 128],
                                     qt[:, h, qs], start=True, stop=True)
                    et = et_pool.tile([128, 512], bf, tag="et", name="et")
                    nc.scalar.activation(et[:, qs], pt[:, qs], EXP, scale=SCALE)
                    if off >= 0:
                        nc.vector.tensor_mul(et[:, qs], et[:, qs],
                                             masks[:, 384:896 - c0])
                    if i == 0:
                        nc.vector.tensor_copy(sums[:], et[:])
                    else:
                        nc.vector.tensor_add(sums[:, qs], sums[:, qs],
                                             et[:, qs])
                    nc.tensor.matmul(ot_ps[:, qs], v_sb[:, i, :], et[:, qs],
                                     start=(i == 0), stop=(i == nsk - 1))
                    if use_filler and i % fill_stride == 2:
                        filler_group(dve_ok=(i < nsk - 4))
                sbc = sbc_pool.tile([128, 512], f32, tag="sbc", name="sbc")
                nc.gpsimd.partition_all_reduce(sbc[:], sums[:], channels=128,
                                               reduce_op=ADD)
                rec = sbc_pool.tile([128, 512], f32, tag="rec", name="rec")
                nc.vector.reciprocal_approx_fast(rec[:], sbc[:])
                nc.vector.tensor_mul(at[:, h, lo:lo + 512], ot_ps[:], rec[:])

            # -------- schedule --------
            # Per block: K, Q0, V, T, Q1, h0, Q2, h1, Q3, h2, h3.
            # qpass(h+1)+rope run BEFORE attn_head(h): the Q-pass matmuls are
            # the PE filler over head h's softmax tail, and rope-Q(h+1) lands
            # on the DVE queue ahead of head h's sums so qt[h+1] is ready.
            reserve = []
            for blk in range(NBLK):
                lo = blk * 512
                sl = slice(lo, lo + 512)
                last = blk == NBLK - 1
                # last block: fewer groups remain (5 held back for the final
                # tail), so spread them across all 4 heads' loops
                fill_stride = 6 if last else 3
                if last:
                    for _ in range(min(6, len(pend))):
                        reserve.append(pend.pop(0))
                k_ps = kpass()
                rope(k_ps, kt[:, sl], sl)
                q_ps = qpass(0)
                rope(q_ps, qt[:, 0, :], sl)
                q_ps = qpass(1)
                rope(q_ps, qt[:, 1, :], sl)
                vpass(blk)
                for h in range(HPC):
                    attn_head(h, blk, blk > 0 and not (last and h == HPC - 1))
                    if h + 2 < HPC:
                        q_ps = qpass(h + 2)
                        rope(q_ps, qt[:, h + 2, :], sl)
                        if h + 2 == HPC - 1 and blk + 1 < NBLK:
                            # Q3 was the last xt reader: start the next
                            # block's xt prefetch now, ~2 heads early
                            nlo = lo + 512
                            for q in range(8):
                                ksl = slice(q * 4, (q + 1) * 4)
                                nc.sync.dma_start(
                                    out=xt_s[:, ksl, :],
                                    in_=xt_r[:, ksl, nlo:nlo + 512])
                    while pend and not last and done[0] < 4 * (h + 1):
                        filler_group(dve_ok=False)
                done[0] = 0
                for m in range(4 * blk, 4 * blk + 4):
                    for np2 in range(HIDDEN // 1024):
                        pend.append((m, np2))

            # tail: reserved pairs (immediately runnable, hide the last
            # softmax tail) then the last block's output projection
            pend[0:0] = reserve
            while pend:
                filler_group(dve_ok=False, split_dma=len(pend) <= 4)

    nc.compile()
    return nc


def _host_prep(hidden_states, position_ids, Wq, Wk, Wv, Wo):
    import ml_dtypes
    bf = ml_dtypes.bfloat16

    X = np.asarray(hidden_states, dtype=np.float32).reshape(S, HIDDEN)
    XT = np.ascontiguousarray(X.T).astype(bf)

    pos = np.asarray(position_ids).reshape(-1)[:S].astype(np.float32)
    inv = (1.0 / (ROPE_BASE ** (np.arange(0, HD, 2, dtype=np.float32) / HD))
           ).astype(np.float32)
    freqs = pos[:, None] * inv[None, :]              # [S, 64]
    cos_h = np.cos(freqs).astype(np.float32)         # [S, 64] (= both halves)
    sin_h = np.sin(freqs).astype(np.float32)
    cosT = np.ascontiguousarray(
        np.concatenate([cos_h, cos_h], axis=1).T).astype(bf)
    sinT = np.concatenate([sin_h, sin_h], axis=1).T
    sinS = np.ascontiguousarray(
        np.concatenate([-sinT[0:64], sinT[64:128]], axis=0)).astype(bf)

    # sliding-window mask [zeros(3x128) | tri | ones(3x128)]: offset o slice
    # starts at (3-o)*128 and covers 512 cols -> c<o zero, c==o tri, c>o ones
    tri = (np.arange(128)[:, None] <= np.arange(128)[None, :]).astype(bf)
    masks = np.concatenate([np.zeros((128, 384), bf), tri,
                            np.ones((128, 384), bf)], axis=1)

    ident = np.eye(128, dtype=bf)

    Wq = np.asarray(Wq, dtype=np.float32)
    Wk = np.asarray(Wk, dtype=np.float32)
    Wv = np.asarray(Wv, dtype=np.float32)
    Wo = np.asarray(Wo, dtype=np.float32)

    in_maps = []
    for c in range(N_CORES):
        in_maps.append({
            "xt": XT,
            "wq": np.ascontiguousarray(Wq[:, c * DQ:(c + 1) * DQ]).astype(bf),
            "wk": np.ascontiguousarray(Wk[:, c * HD:(c + 1) * HD]).astype(bf),
            "wv": np.ascontiguousarray(Wv[:, c * HD:(c + 1) * HD]).astype(bf),
            "wo": np.ascontiguousarray(Wo[c * DQ:(c + 1) * DQ, :]).astype(bf),
            "cosT": cosT,
            "sinS": sinS,
            "masks": masks,
            "ident": ident,
        })
    return in_maps


def kernel(hidden_states, position_ids, Wq, Wk, Wv, Wo, _run_opts=None):
    from concourse.bass_utils import run_bass_kernel_spmd

    if "nc" not in _CACHE:
        _CACHE["nc"] = _build()
    nc = _CACHE["nc"]

    in_maps = _host_prep(hidden_states, position_ids, Wq, Wk, Wv, Wo)
    opts = dict(_run_opts or {})
    res = run_bass_kernel_spmd(nc, in_maps, core_ids=list(range(N_CORES)), **opts)
    _CACHE["last_result"] = res

    out = res.results[0]["out"].astype(np.float64)
    for c in range(1, N_CORES):
        out += res.results[c]["out"].astype(np.float64)
    return out.astype(np.float32).reshape(1, S, HIDDEN)


# revision 32
# speedup vs baseline: 1.0323x; 1.0323x over previous
"""LlamaAttention (B=1, S=2048, H=4096, 32 heads / 8 KV heads) on 8 TRN2 NeuronCores.

Sharding: tensor-parallel over heads. Core c owns Q heads [4c, 4c+4) and KV head c
(Wq/Wk/Wv column shards, Wo row shard). Each core computes a full [S, H] partial
output in bf16; the host sums the 8 partials (the all-reduce for row-sharded Wo).

vs baseline (894us -> ~404us):
- all matmul operands bf16 (half DMA/SBUF traffic, same PE rate as fp32r);
  error 8e-3 vs the 2e-2 gate. All weights + rope tables SBUF-resident
  (baseline re-streamed Wq 4x); xt block-resident, next block prefetched as
  soon as the last projection pass has read the buffer.
- per-head projection passes cut concurrent PSUM banks so attention gets
  pt x2 + ot x2 banks; out-projection runs as "filler" matmuls interleaved
  into the NEXT block's attention loops + drains, hiding every softmax tail
  (gpsimd partition-reduce 3.5us + reciprocal_approx_fast + normalize) and
  keeping the PE HAM clock at 8/8 (baseline lost 300us to 1.2 GHz throttle).
- qpass(h+1)+rope issued before attn_head(h) so rope lands on the DVE queue
  ahead of head h's softmax sums; filler PSUM-evacs routed ScalarE/VectorE
  by phase so neither engine's backlog stalls the ps_o bank rotation.
- diagonal causal tiles computed at partial width (queries below the band
  see nothing); dummy warm-up matmuls spin the PE clock up during the
  initial DMA fill; 6 filler groups reserved to cover the last head's tail.
"""

import numpy as np

HIDDEN = 4096
N_HEADS = 32
N_KV = 8
HD = 128
S = 2048
N_CORES = 8
HPC = N_HEADS // N_CORES          # 4 Q heads per core
DQ = HPC * HD                     # 512 q columns per core
ROPE_BASE = 10000.0
SCALE = 1.0 / float(np.sqrt(HD))

NBLK = S // 512                   # 4 sq blocks of 512
NSK = S // 128                    # 16 sk tiles of 128
KT = HIDDEN // 128                # 32 contraction tiles

_CACHE = {}


def _build():
    import concourse.bass as bass
    import concourse.tile as tile
    from concourse import bacc, mybir
    import concourse.bass_isa as bass_isa

    f32 = mybir.dt.float32
    bf = mybir.dt.bfloat16
    EXP = mybir.ActivationFunctionType.Exp
    CPY = mybir.ActivationFunctionType.Copy
    ADD = bass_isa.ReduceOp.add

    nc = bacc.Bacc("TRN2", target_bir_lowering=False, debug=False,
                   num_devices=N_CORES)

    xt_d = nc.dram_tensor("xt", [HIDDEN, S], bf, kind="ExternalInput").ap()
    wq_d = nc.dram_tensor("wq", [HIDDEN, DQ], bf, kind="ExternalInput").ap()
    wk_d = nc.dram_tensor("wk", [HIDDEN, HD], bf, kind="ExternalInput").ap()
    wv_d = nc.dram_tensor("wv", [HIDDEN, HD], bf, kind="ExternalInput").ap()
    wo_d = nc.dram_tensor("wo", [DQ, HIDDEN], bf, kind="ExternalInput").ap()
    cos_d = nc.dram_tensor("cosT", [HD, S], bf, kind="ExternalInput").ap()
    sin_d = nc.dram_tensor("sinS", [HD, S], bf, kind="ExternalInput").ap()
    msk_d = nc.dram_tensor("masks", [128, 896], bf, kind="ExternalInput").ap()
    idn_d = nc.dram_tensor("ident", [128, 128], bf, kind="ExternalInput").ap()
    out_d = nc.dram_tensor("out", [S, HIDDEN], bf, kind="ExternalOutput").ap()

    with tile.TileContext(nc) as tc:
        from contextlib import ExitStack
        with ExitStack() as ctx:
            ep = ctx.enter_context
            consts = ep(tc.tile_pool(name="consts", bufs=1))
            main = ep(tc.tile_pool(name="main", bufs=1))
            tmp_pool = ep(tc.tile_pool(name="tmpp", bufs=2))
            et_pool = ep(tc.tile_pool(name="etp", bufs=8))
            sums_pool = ep(tc.tile_pool(name="sumsp", bufs=3))
            sbc_pool = ep(tc.tile_pool(name="sbcp", bufs=3))
            vt_pool = ep(tc.tile_pool(name="vtp", bufs=2))
            osb_pool = ep(tc.tile_pool(name="osbp", bufs=4))
            ps_kv = ep(tc.tile_pool(name="pskv", bufs=1, space="PSUM"))
            ps_q = ep(tc.tile_pool(name="psq", bufs=1, space="PSUM"))
            ps_pt = ep(tc.tile_pool(name="pspt", bufs=2, space="PSUM"))
            ps_ot = ep(tc.tile_pool(name="psot", bufs=2, space="PSUM"))
            ps_o = ep(tc.tile_pool(name="pso", bufs=2, space="PSUM"))

            # ---- resident constants / weights (order = DMA issue order) ----
            wk_s = consts.tile([128, KT, HD], bf)
            wv_s = consts.tile([128, KT, HD], bf)
            wq_s = consts.tile([128, KT, DQ], bf)
            wo_s = consts.tile([128, HPC, HIDDEN], bf)
            cosT = consts.tile([HD, S], bf)
            sinS = consts.tile([HD, S], bf)
            masks = consts.tile([128, 896], bf)
            ident = consts.tile([128, 128], bf)
            xt_s = main.tile([128, KT, 512], bf)
            kt = main.tile([128, S], bf)
            v_sb = main.tile([128, NSK, 128], bf)
            qt = main.tile([128, HPC, 512], bf)
            at = main.tile([128, HPC, S], bf)

            wk_r = wk_d.rearrange("(kt p) m -> p kt m", p=128)
            wv_r = wv_d.rearrange("(kt p) m -> p kt m", p=128)
            wq_r = wq_d.rearrange("(kt p) m -> p kt m", p=128)
            wo_r = wo_d.rearrange("(hh p) m -> p hh m", p=128)

            # K weights + first xt block first so the K pass starts ASAP;
            # wq/wv next (Q0/V pass dependencies), rope tables and Wo later.
            xt_r = xt_d.rearrange("(kt p) s -> p kt s", p=128)
            for q in range(4):
                ksl = slice(q * 8, (q + 1) * 8)
                nc.sync.dma_start(out=wk_s[:, ksl, :], in_=wk_r[:, ksl, :])
            # interleaved so each consumer starts as early as possible:
            # K pass <- wk + xt; ropes <- tables; Q0 <- wq chunks; V <- wv
            for q in range(4):
                ksl = slice(q * 4, (q + 1) * 4)
                nc.sync.dma_start(out=xt_s[:, ksl, :],
                                  in_=xt_r[:, ksl, 0:512])
            nc.sync.dma_start(out=cosT, in_=cos_d)
            nc.sync.dma_start(out=sinS, in_=sin_d)
            nc.sync.dma_start(out=masks, in_=msk_d)
            nc.sync.dma_start(out=ident, in_=idn_d)
            for q in range(2):
                ksl = slice(q * 8, (q + 1) * 8)
                nc.sync.dma_start(out=wq_s[:, ksl, :], in_=wq_r[:, ksl, :])
            for q in range(4, 8):
                ksl = slice(q * 4, (q + 1) * 4)
                nc.sync.dma_start(out=xt_s[:, ksl, :],
                                  in_=xt_r[:, ksl, 0:512])
            for q in range(2, 4):
                ksl = slice(q * 8, (q + 1) * 8)
                nc.sync.dma_start(out=wq_s[:, ksl, :], in_=wq_r[:, ksl, :])
            for q in range(4):
                ksl = slice(q * 8, (q + 1) * 8)
                nc.sync.dma_start(out=wv_s[:, ksl, :], in_=wv_r[:, ksl, :])
            for q in range(4):
                nsl = slice(q * 1024, (q + 1) * 1024)
                nc.sync.dma_start(out=wo_s[:, :, nsl], in_=wo_r[:, :, nsl])

            # PE warm-up: dummy matmuls on an un-initialized tile get the HAM
            # clock gate to 8/8 before the first real matmul arrives.
            wrm = main.tile([128, 512], bf)
            nc.vector.memset(wrm[:], 0.0)
            for w in range(16):
                wps = ps_pt.tile([128, 512], f32, tag="pt", name="wps")
                nc.tensor.matmul(wps[:], wrm[:, 0:128], wrm[:],
                                 start=True, stop=True)

            # ---- out-projection filler machinery (n-pair granularity) ----
            pend = []
            done = [0]

            def filler_group(dve_ok=True, split_dma=False):
                """One filler unit: out rows m*128..+128, cols np2*1024..+1024
                (8 matmuls -> 2 PSUM groups -> one [128,1024] bf16 store).
                Evac engine choice matters: inside attention loops ScalarE is
                busy with exp (use DVE for one half); in the drains the DVE
                holds the softmax tail (recip/at-mul) so use ScalarE only."""
                if not pend:
                    return
                m, np2 = pend.pop(0)
                osb = osb_pool.tile([128, 1024], bf, tag="osb", name="osb")
                for half in range(2):
                    n = np2 * 2 + half
                    o_ps = ps_o.tile([128, 512], f32, tag="ops", name="ops")
                    for hh in range(HPC):
                        nc.tensor.matmul(o_ps[:],
                                         at[:, hh, m * 128:(m + 1) * 128],
                                         wo_s[:, hh, n * 512:(n + 1) * 512],
                                         start=(hh == 0), stop=(hh == HPC - 1))
                    if half == 1 and dve_ok:
                        nc.vector.tensor_copy(osb[:, 512:1024], o_ps[:])
                    else:
                        nc.scalar.activation(osb[:, half * 512:(half + 1) * 512],
                                             o_ps[:], CPY)
                    if split_dma:
                        nc.sync.dma_start(
                            out=out_d[m * 128:(m + 1) * 128,
                                      n * 512:(n + 1) * 512],
                            in_=osb[:, half * 512:(half + 1) * 512])
                if not split_dma:
                    nc.sync.dma_start(
                        out=out_d[m * 128:(m + 1) * 128,
                                  np2 * 1024:(np2 + 1) * 1024],
                        in_=osb[:])
                done[0] += 1

            # ---- per-block building blocks ----
            def rope(ps, dst, sl):
                """dst = rope(ps) (bf16 out) using cosT/sinS tables."""
                t = tmp_pool.tile([128, 512], f32, tag="ropet", name="ropet")
                nc.vector.tensor_mul(t[0:64, :], ps[64:128, :], sinS[0:64, sl])
                nc.vector.tensor_mul(t[64:128, :], ps[0:64, :], sinS[64:128, sl])
                nc.vector.tensor_mul(dst, ps[:], cosT[:, sl])
                nc.vector.tensor_add(dst, dst, t[:])

            def kpass():
                k_ps = ps_kv.tile([128, 512], f32, tag="kv", name="kps")
                for k in range(KT):
                    nc.tensor.matmul(k_ps[:], wk_s[:, k, :], xt_s[:, k, :],
                                     start=(k == 0), stop=(k == KT - 1))
                return k_ps

            def qpass(h):
                q_ps = ps_q.tile([128, 512], f32, tag="qps", name="qps")
                for k in range(KT):
                    nc.tensor.matmul(q_ps[:], wq_s[:, k, h * 128:(h + 1) * 128],
                                     xt_s[:, k, :],
                                     start=(k == 0), stop=(k == KT - 1))
                return q_ps

            def vpass(blk):
                v_ps = ps_kv.tile([128, 512], f32, tag="kv", name="vps")
                for k in range(KT):
                    nc.tensor.matmul(v_ps[:], wv_s[:, k, :], xt_s[:, k, :],
                                     start=(k == 0), stop=(k == KT - 1))
                vt_sb = vt_pool.tile([128, 512], bf, tag="vt", name="vt")
                nc.scalar.activation(vt_sb[:], v_ps[:], CPY)
                vtr = ps_kv.tile([128, 4, 128], bf, tag="kv", name="vtr")
                for t in range(4):
                    nc.tensor.transpose(vtr[:, t, :],
                                        vt_sb[:, t * 128:(t + 1) * 128],
                                        ident[:])
                nc.scalar.activation(v_sb[:, blk * 4:(blk + 1) * 4, :], vtr[:],
                                     CPY)

            def attn_head(h, blk, use_filler):
                lo = blk * 512
                nsk = 4 * (blk + 1)
                ot_ps = ps_ot.tile([128, 512], f32, tag="ot", name="otps")
                sums = sums_pool.tile([128, 512], f32, tag="sums", name="sums")
                for i in range(nsk):
                    off = i - 4 * blk
                    # diagonal-band tiles: queries below off*128 see nothing
                    c0 = off * 128 if off > 0 else 0
                    qs = slice(c0, 512)
                    pt = ps_pt.tile([128, 512], f32, tag="pt", name="pt")
                    nc.tensor.matmul(pt[:, qs], kt[:, i * 128:(i + 1) * 128],
                                     qt[:, h, qs], start=True, stop=True)
                    et = et_pool.tile([128, 512], bf, tag="et", name="et")
                    nc.scalar.activation(et[:, qs], pt[:, qs], EXP, scale=SCALE)
                    if off >= 0:
                        nc.vector.tensor_mul(et[:, qs], et[:, qs],
                                             masks[:, 384:896 - c0])
                    if i == 0:
                        nc.vector.tensor_copy(sums[:], et[:])
                    else:
                        nc.vector.tensor_add(sums[:, qs], sums[:, qs],
                                             et[:, qs])
                    nc.tensor.matmul(ot_ps[:, qs], v_sb[:, i, :], et[:, qs],
                                     start=(i == 0), stop=(i == nsk - 1))
                    if use_filler and i % fill_stride == 2:
                        filler_group(dve_ok=(i < nsk - 4))
                sbc = sbc_pool.tile([128, 512], f32, tag="sbc", name="sbc")
                nc.gpsimd.partition_all_reduce(sbc[:], sums[:], channels=128,
                                               reduce_op=ADD)
                rec = sbc_pool.tile([128, 512], f32, tag="rec", name="rec")
                nc.vector.reciprocal_approx_fast(rec[:], sbc[:])
                nc.vector.tensor_mul(at[:, h, lo:lo + 512], ot_ps[:], rec[:])

            # -------- schedule --------
            # Per block: K, Q0, V, T, Q1, h0, Q2, h1, Q3, h2, h3.
            # qpass(h+1)+rope run BEFORE attn_head(h): the Q-pass matmuls are
            # the PE filler over head h's softmax tail, and rope-Q(h+1) lands
            # on the DVE queue ahead of head h's sums so qt[h+1] is ready.
            reserve = []
            for blk in range(NBLK):
                lo = blk * 512
                sl = slice(lo, lo + 512)
                last = blk == NBLK - 1
                # last block: fewer groups remain (5 held back for the final
                # tail), so spread them across all 4 heads' loops
                fill_stride = 6 if last else 3
                if last:
                    for _ in range(min(6, len(pend))):
                        reserve.append(pend.pop(0))
                k_ps = kpass()
                rope(k_ps, kt[:, sl], sl)
                q_ps = qpass(0)
                rope(q_ps, qt[:, 0, :], sl)
                vpass(blk)
                q_ps = qpass(1)
                rope(q_ps, qt[:, 1, :], sl)
                for h in range(HPC):
                    attn_head(h, blk, blk > 0 and not (last and h == HPC - 1))
                    if h + 2 < HPC:
                        q_ps = qpass(h + 2)
                        rope(q_ps, qt[:, h + 2, :], sl)
                        if h + 2 == HPC - 1 and blk + 1 < NBLK:
                            # Q3 was the last xt reader: start the next
                            # block's xt prefetch now, ~2 heads early
                            nlo = lo + 512
                            for q in range(8):
                                ksl = slice(q * 4, (q + 1) * 4)
                                nc.sync.dma_start(
                                    out=xt_s[:, ksl, :],
                                    in_=xt_r[:, ksl, nlo:nlo + 512])
                    while pend and not last and done[0] < 4 * (h + 1):
                        filler_group(dve_ok=False)
                done[0] = 0
                for m in range(4 * blk, 4 * blk + 4):
                    for np2 in range(HIDDEN // 1024):
                        pend.append((m, np2))

            # tail: reserved pairs (immediately runnable, hide the last
            # softmax tail) then the last block's output projection
            pend[0:0] = reserve
            while pend:
                filler_group(dve_ok=False, split_dma=len(pend) <= 4)

    nc.compile()
    return nc


def _host_prep(hidden_states, position_ids, Wq, Wk, Wv, Wo):
    import ml_dtypes
    bf = ml_dtypes.bfloat16

    X = np.asarray(hidden_states, dtype=np.float32).reshape(S, HIDDEN)
    XT = np.ascontiguousarray(X.T).astype(bf)

    pos = np.asarray(position_ids).reshape(-1)[:S].astype(np.float32)
    inv = (1.0 / (ROPE_BASE ** (np.arange(0, HD, 2, dtype=np.float32) / HD))
           ).astype(np.float32)
    freqs = pos[:, None] * inv[None, :]              # [S, 64]
    cos_h = np.cos(freqs).astype(np.float32)         # [S, 64] (= both halves)
    sin_h = np.sin(freqs).astype(np.float32)
    cosT = np.ascontiguousarray(
        np.concatenate([cos_h, cos_h], axis=1).T).astype(bf)
    sinT = np.concatenate([sin_h, sin_h], axis=1).T
    sinS = np.ascontiguousarray(
        np.concatenate([-sinT[0:64], sinT[64:128]], axis=0)).astype(bf)

    # sliding-window mask [zeros(3x128) | tri | ones(3x128)]: offset o slice
    # starts at (3-o)*128 and covers 512 cols -> c<o zero, c==o tri, c>o ones
    tri = (np.arange(128)[:, None] <= np.arange(128)[None, :]).astype(bf)
    masks = np.concatenate([np.zeros((128, 384), bf), tri,
                            np.ones((128, 384), bf)], axis=1)

    ident = np.eye(128, dtype=bf)

    Wq = np.asarray(Wq, dtype=np.float32)
    Wk = np.asarray(Wk, dtype=np.float32)
    Wv = np.asarray(Wv, dtype=np.float32)
    Wo = np.asarray(Wo, dtype=np.float32)

    in_maps = []
    for c in range(N_CORES):
        in_maps.append({
            "xt": XT,
            "wq": np.ascontiguousarray(Wq[:, c * DQ:(c + 1) * DQ]).astype(bf),
            "wk": np.ascontiguousarray(Wk[:, c * HD:(c + 1) * HD]).astype(bf),
            "wv": np.ascontiguousarray(Wv[:, c * HD:(c + 1) * HD]).astype(bf),
            "wo": np.ascontiguousarray(Wo[c * DQ:(c + 1) * DQ, :]).astype(bf),
            "cosT": cosT,
            "sinS": sinS,
            "masks": masks,
            "ident": ident,
        })
    return in_maps


def kernel(hidden_states, position_ids, Wq, Wk, Wv, Wo, _run_opts=None):
    from concourse.bass_utils import run_bass_kernel_spmd

    if "nc" not in _CACHE:
        _CACHE["nc"] = _build()
    nc = _CACHE["nc"]

    in_maps = _host_prep(hidden_states, position_ids, Wq, Wk, Wv, Wo)
    opts = dict(_run_opts or {})
    res = run_bass_kernel_spmd(nc, in_maps, core_ids=list(range(N_CORES)), **opts)
    _CACHE["last_result"] = res

    out = res.results[0]["out"].astype(np.float64)
    for c in range(1, N_CORES):
        out += res.results[c]["out"].astype(np.float64)
    return out.astype(np.float32).reshape(1, S, HIDDEN)
